# revision 11
# baseline (speedup 1.0000x reference)
"""Causal self-attention (QKV GEMM + RoPE + causal softmax attention + output
projection) for Trainium2, sharded over 8 NeuronCores.

Sharding: tensor-parallel over heads (2 heads/core). Each core computes the
QKV projections for its heads (full token range), RoPE, causal attention, and
a partial output projection over its heads' channels; the host sums the 8
partial projections (the only cross-core reduction) and reshapes.

Matmul operands are fp16 (full-rate PE); accumulation is fp32 in PSUM.
Inputs (x, w_attn, w_proj, mask) are pre-cast to fp16 on the host to halve
HBM input traffic; the partial outputs are written back as fp16 as well.
The softmax denominator is accumulated on the Vector engine (one PE matmul
per 512-query chunk instead of one per key tile), score matmuls are issued
a few tiles ahead of the attn@V matmuls to hide the exp() latency, and the
output projection is interleaved with attention per query chunk so its DMA
hides under compute.
"""

import os
import sys

import numpy as np


def _ensure_concourse():
    try:
        import concourse.bass  # noqa: F401
        return
    except ImportError:
        pass
    for p in (
        "/opt/trn_rl_repo",
        os.path.expanduser("~/.axon_site/_ro/trn_rl_repo"),
        "/root/.axon_site/_ro/trn_rl_repo",
    ):
        if os.path.isdir(p) and p not in sys.path:
            sys.path.insert(0, p)
    import concourse.bass  # noqa: F401


# Problem shape (hardcoded per contract)
B, T, C, H = 2, 2048, 2048, 16
D, RD = 128, 64
NCORES = 8
HPC = H // NCORES          # heads per core = 2
BT = B * T                 # 4096
P = 128
MT = T // P                # 16 token tiles per batch
KTC = C // P               # 16 contraction tiles over C
KH = KTC // 2              # kt tiles per half
FPC = 3 * HPC * D          # 768 qkv features per core
NQ = 512                   # query chunk
NJ = T // NQ               # 4 query chunks per instance
SCALE = 1.0 / float(np.sqrt(D))
DEPTH = 3                  # how many score matmuls run ahead of attn@V

_PROGRAM = None


def _build_program():
    _ensure_concourse()
    from contextlib import ExitStack

    import concourse.bacc as bacc
    import concourse.mybir as mybir
    import concourse.tile as tile
    from concourse.alu_op_type import AluOpType
    from concourse.masks import make_identity

    F32 = mybir.dt.float32
    F16 = mybir.dt.float16
    EXP = mybir.ActivationFunctionType.Exp
    MUL = AluOpType.mult
    SUB = AluOpType.subtract
    ADD = AluOpType.add
    PSUM = "PSUM"

    nc = bacc.Bacc("TRN2", target_bir_lowering=False, debug=False,
                   num_devices=NCORES)

    xt_d = nc.dram_tensor("xt", [C, BT], F16, kind="ExternalInput").ap()
    w_d = nc.dram_tensor("wqkv", [P, KTC * FPC], F16, kind="ExternalInput").ap()
    cos_d = nc.dram_tensor("cosw", [P, (BT // P) * RD], F32, kind="ExternalInput").ap()
    sin_d = nc.dram_tensor("sinw", [P, (BT // P) * RD], F32, kind="ExternalInput").ap()
    msk_d = nc.dram_tensor("maskd", [P, 4 * NQ], F16, kind="ExternalInput").ap()
    wp_d = nc.dram_tensor("wproj", [P, HPC * C], F16, kind="ExternalInput").ap()
    out_d = nc.dram_tensor("outp", [BT, C], F16, kind="ExternalOutput").ap()

    xt_r = xt_d.rearrange("(kt p) t -> p kt t", p=P)

    with tile.TileContext(nc) as tc, ExitStack() as gctx:
        ep = gctx.enter_context

        const = ep(tc.tile_pool(name="const", bufs=1))
        msk_sb = const.tile([P, 4 * NQ], F16, tag="msk")
        cos_sb = const.tile([P, (BT // P) * RD], F32, tag="cos")
        sin_sb = const.tile([P, (BT // P) * RD], F32, tag="sin")
        ident = const.tile([P, P], F16, tag="ident")
        ones_cf = const.tile([P, 1], F32, tag="ones_cf")
        ones_c = const.tile([P, 1], F16, tag="ones_c")
        wp_sb = const.tile([P, HPC * C], F16, tag="wp")

        wpool = ep(tc.tile_pool(name="wqkv", bufs=1))
        w_sb0 = wpool.tile([P, KH * FPC], F16, tag="w0")
        w_sb1 = wpool.tile([P, KH * FPC], F16, tag="w1")

        # Weight half 0 first: it gates the very first matmul. Remaining
        # constants are issued right after the second x-column DMA (below) so
        # their transfers don't delay the pipeline start.
        nc.gpsimd.dma_start(out=w_sb0[:], in_=w_d[:, :KH * FPC])

        def emit_late_consts():
            nc.gpsimd.dma_start(out=w_sb1[:], in_=w_d[:, KH * FPC:])
            nc.gpsimd.dma_start(out=cos_sb[:], in_=cos_d)
            nc.gpsimd.dma_start(out=sin_sb[:], in_=sin_d)
            nc.gpsimd.dma_start(out=msk_sb[:], in_=msk_d)
            nc.gpsimd.dma_start(out=wp_sb[:], in_=wp_d)
            make_identity(nc, ident[:])
            nc.vector.memset(ones_cf[:], 1.0)
            nc.vector.tensor_copy(ones_c[:], ones_cf[:])

        def w_at(kt):
            if kt < KH:
                return w_sb0[:, kt * FPC:(kt + 1) * FPC]
            return w_sb1[:, (kt - KH) * FPC:(kt - KH + 1) * FPC]

        qkt_pool = ep(tc.tile_pool(name="qkt", bufs=1))
        v_pool = ep(tc.tile_pool(name="v", bufs=1))
        yt_pool = ep(tc.tile_pool(name="yt", bufs=1))
        yt_all = yt_pool.tile([P, B * HPC * T], F16, tag="yt")
        xcol = ep(tc.tile_pool(name="xcol", bufs=2))
        rotp = ep(tc.tile_pool(name="rot", bufs=3))
        tmpp = ep(tc.tile_pool(name="tmp", bufs=2))

        consts_emitted = False

        for b in range(B):
            qkT = qkt_pool.tile([P, 4 * T], F16, tag="qkT")
            v_sb = v_pool.tile([P, MT * HPC * D], F16, tag="v")

            # ---- Phase A: QKV GEMM + RoPE + transpose of Q,K ----
            with ExitStack() as actx:
                ap = actx.enter_context
                ps5 = ap(tc.tile_pool(name="ps5", bufs=3, space=PSUM))
                ps2 = ap(tc.tile_pool(name="ps2", bufs=3, space=PSUM))
                pst = ap(tc.tile_pool(name="pst", bufs=2, space=PSUM))

                m0 = 0
                # small first group: its x-column DMA is 3x smaller, so the
                # first matmul issues sooner after kernel start
                for g in ([1] + [3] * 5):
                    tsl = slice(b * T + m0 * P, b * T + (m0 + g) * P)
                    p5s = [ps5.tile([P, 512], F32, tag="p5", name=f"p5_{b}_{m0}_{i}")
                           for i in range(g)]
                    p2s = [ps2.tile([P, 256], F32, tag="p2", name=f"p2_{b}_{m0}_{i}")
                           for i in range(g)]
                    for half in range(2):
                        xc = xcol.tile([P, KH, P * g], F16, tag="xc")
                        nc.gpsimd.dma_start(
                            out=xc[:],
                            in_=xt_r[:, half * KH:(half + 1) * KH, tsl])
                        if not consts_emitted and half == 1:
                            emit_late_consts()
                            consts_emitted = True
                        for mi in range(g):
                            for k8 in range(KH):
                                kt = half * KH + k8
                                lhsT = xc[:, k8, mi * P:(mi + 1) * P]
                                wsl = w_at(kt)
                                nc.tensor.matmul(
                                    p5s[mi][:], lhsT, wsl[:, :512],
                                    start=(kt == 0), stop=(kt == KTC - 1))
                                nc.tensor.matmul(
                                    p2s[mi][:], lhsT, wsl[:, 512:],
                                    start=(kt == 0), stop=(kt == KTC - 1))
                    for mi in range(g):
                        m = m0 + mi
                        gm = b * MT + m
                        p5 = p5s[mi]
                        p2 = p2s[mi]
                        # RoPE on the q|k half (psum chunk p5), writes rot
                        rot = rotp.tile([P, 512], F16, tag="rot")
                        p3 = p5[:].rearrange("p (blk two d) -> p blk two d",
                                             two=2, d=RD)
                        re_, im_ = p3[:, :, 0, :], p3[:, :, 1, :]
                        r3 = rot[:].rearrange("p (blk two d) -> p blk two d",
                                              two=2, d=RD)
                        cosb = (cos_sb[:, gm * RD:(gm + 1) * RD]
                                .unsqueeze(1).broadcast_to([P, 4, RD]))
                        sinb = (sin_sb[:, gm * RD:(gm + 1) * RD]
                                .unsqueeze(1).broadcast_to([P, 4, RD]))
                        t1 = tmpp.tile([P, 256], F32, tag="t1")
                        t2 = tmpp.tile([P, 256], F32, tag="t2")
                        t1v = t1[:].rearrange("p (blk d) -> p blk d", d=RD)
                        t2v = t2[:].rearrange("p (blk d) -> p blk d", d=RD)
                        nc.vector.tensor_tensor(t1v, re_, cosb, MUL)
                        nc.vector.tensor_tensor(t2v, im_, sinb, MUL)
                        nc.vector.tensor_tensor(r3[:, :, 0, :], t1v, t2v, SUB)
                        t3 = tmpp.tile([P, 256], F32, tag="t3")
                        t4 = tmpp.tile([P, 256], F32, tag="t4")
                        t3v = t3[:].rearrange("p (blk d) -> p blk d", d=RD)
                        t4v = t4[:].rearrange("p (blk d) -> p blk d", d=RD)
                        nc.vector.tensor_tensor(t3v, re_, sinb, MUL)
                        nc.vector.tensor_tensor(t4v, im_, cosb, MUL)
                        nc.vector.tensor_tensor(r3[:, :, 1, :], t3v, t4v, ADD)
                        # V eviction
                        nc.scalar.copy(v_sb[:, m * HPC * D:(m + 1) * HPC * D],
                                       p2[:])
                        # Transpose rotated q,k into [d, t] layout; evict on
                        # DVE so the scalar engine enters attention unqueued
                        for hb in range(4):
                            tp = pst.tile([P, P], F16, tag="tp")
                            nc.tensor.transpose(
                                tp[:], rot[:, hb * P:(hb + 1) * P], ident[:])
                            nc.vector.tensor_copy(
                                qkT[:, hb * T + m * P:(hb * T + (m + 1) * P)],
                                tp[:])
                    m0 += g

            # ---- Phase B+C: causal attention + interleaved output proj ----
            with ExitStack() as bctx:
                bp = bctx.enter_context
                attnp = bp(tc.tile_pool(name="attn", bufs=4))
                accp = bp(tc.tile_pool(name="acc", bufs=2))
                rcpp = bp(tc.tile_pool(name="rcp", bufs=2))
                repp = bp(tc.tile_pool(name="rep", bufs=2))
                orowp = bp(tc.tile_pool(name="orow", bufs=2))
                pss = bp(tc.tile_pool(name="pss", bufs=2, space=PSUM))
                psy = bp(tc.tile_pool(name="psy", bufs=2, space=PSUM))
                pso = bp(tc.tile_pool(name="pso", bufs=2, space=PSUM))

                def finalize(pend):
                    # softmax normalization, pipelined one chunk behind
                    inst, j, y_ps, s_ps = pend
                    rcp = rcpp.tile([1, NQ], F32, tag="rc",
                                    name=f"rc_{inst}_{j}")
                    with nc.allow_low_precision(reason="softmax recip"):
                        nc.vector.reciprocal_approx_fast(out=rcp[:],
                                                         in_=s_ps[:])
                    reps = repp.tile([P, NQ], F32, tag="rs",
                                     name=f"rs_{inst}_{j}")
                    nc.gpsimd.partition_broadcast(reps[:], rcp[:], channels=P)
                    nc.vector.tensor_tensor(
                        yt_all[:, inst * T + j * NQ: inst * T + (j + 1) * NQ],
                        y_ps[:], reps[:], MUL)

                def proj_chunk(j):
                    # output projection for the 4 token tiles of chunk j
                    for mi in range(4):
                        m = j * 4 + mi
                        orow = orowp.tile([P, C], F16, tag="orow",
                                          name=f"orow_{b}_{m}")
                        for oc in range(4):
                            op = pso.tile([P, 512], F32, tag="op",
                                          name=f"op_{b}_{m}_{oc}")
                            for h in range(HPC):
                                nc.tensor.matmul(
                                    op[:],
                                    yt_all[:, (b * HPC + h) * T + m * P:
                                           (b * HPC + h) * T + (m + 1) * P],
                                    wp_sb[:, h * C + oc * 512:
                                          h * C + (oc + 1) * 512],
                                    start=(h == 0), stop=(h == HPC - 1))
                            osl = orow[:, oc * 512:(oc + 1) * 512]
                            if oc % 2 == 0:
                                nc.scalar.copy(osl, op[:])
                            else:
                                nc.vector.tensor_copy(osl, op[:])
                        nc.sync.dma_start(
                            out=out_d[(b * MT + m) * P:(b * MT + m + 1) * P, :],
                            in_=orow[:])

                pending = None
                pending_proj = None
                for j in range(NJ):
                    for h in range(HPC):
                        inst = b * HPC + h
                        nkt = 4 * (j + 1)
                        npair = nkt // 2
                        y_ps = psy.tile([P, NQ], F32, tag="y",
                                        name=f"y_{inst}_{j}")
                        acc = accp.tile([P, NQ], F16, tag="acc",
                                        name=f"acc_{inst}_{j}")
                        ats = []

                        def y_pair(pp):
                            at2 = ats[pp]
                            for half in range(2):
                                ky = 2 * pp + half
                                nc.tensor.matmul(
                                    y_ps[:],
                                    v_sb[:, ky * HPC * D + h * D:
                                         ky * HPC * D + (h + 1) * D],
                                    at2[:, half * NQ:(half + 1) * NQ],
                                    start=(ky == 0), stop=(ky == nkt - 1))

                        for pp in range(npair):
                            # two key tiles share one psum pair so exp/mask/
                            # accumulate amortize their per-op overhead
                            sc2 = pss.tile([P, 2 * NQ], F32, tag="sc",
                                           name=f"sc_{inst}_{j}_{pp}")
                            for half in range(2):
                                kt = 2 * pp + half
                                nc.tensor.matmul(
                                    sc2[:, half * NQ:(half + 1) * NQ],
                                    qkT[:, (2 + h) * T + kt * P:
                                        (2 + h) * T + (kt + 1) * P],
                                    qkT[:, h * T + j * NQ:
                                        h * T + (j + 1) * NQ],
                                    start=True, stop=True)
                            at2 = attnp.tile([P, 2 * NQ], F16, tag="at",
                                             name=f"at_{inst}_{j}_{pp}")
                            nc.scalar.activation(at2[:], sc2[:], EXP,
                                                 scale=SCALE)
                            if 2 * pp >= nkt - 4:
                                i = 2 * pp - (nkt - 4)
                                nc.vector.tensor_tensor(
                                    at2[:], at2[:],
                                    msk_sb[:, i * NQ:(i + 2) * NQ], MUL)
                            if pp == 0:
                                nc.vector.tensor_copy(acc[:], at2[:, :NQ])
                            else:
                                nc.vector.tensor_tensor(acc[:], acc[:],
                                                        at2[:, :NQ], ADD)
                            nc.vector.tensor_tensor(acc[:], acc[:],
                                                    at2[:, NQ:], ADD)
                            ats.append(at2)
                            if pp >= 1:
                                y_pair(pp - 1)
                            if pp == 0 and pending is not None:
                                finalize(pending)
                                pending = None
                            if pp == min(1, npair - 1) and \
                                    pending_proj is not None:
                                proj_chunk(pending_proj)
                                pending_proj = None
                        y_pair(npair - 1)
                        # softmax denominator: single column-sum matmul over
                        # the Pool-accumulated exp values
                        s_ps = pss.tile([1, NQ], F32, tag="sc",
                                        name=f"s_{inst}_{j}")
                        nc.tensor.matmul(s_ps[:], ones_c[:], acc[:],
                                         start=True, stop=True)
                        pending = (inst, j, y_ps, s_ps)
                    pending_proj = j
                # drain
                if pending is not None:
                    finalize(pending)
                    pending = None
                if pending_proj is not None:
                    proj_chunk(pending_proj)
                    pending_proj = None

    nc.compile()
    return nc


def _perm(rows):
    return np.concatenate([rows[0::2], rows[1::2]], axis=0)


def _host_inputs(x, mask, freqs_cos, freqs_sin, w_attn, w_proj):
    f32 = np.float32
    f16 = np.float16
    x = np.asarray(x, f32)
    mask = np.asarray(mask)
    fc = np.asarray(freqs_cos, f32)
    fs = np.asarray(freqs_sin, f32)
    w_attn = np.asarray(w_attn, f32)
    w_proj = np.asarray(w_proj, f32)

    xT = np.ascontiguousarray(x.reshape(BT, C).T.astype(f16))

    def rows_arrange(a):  # [BT, RD] -> [P, (BT//P)*RD]
        return np.ascontiguousarray(
            a.reshape(BT // P, P, RD).transpose(1, 0, 2).reshape(P, -1))

    cosw = rows_arrange(np.concatenate([fc] * B, axis=0))
    sinw = rows_arrange(np.concatenate([fs] * B, axis=0))

    maskd = np.concatenate(
        [mask[0:NQ, i * P:(i + 1) * P].T.astype(f16) for i in range(4)],
        axis=1)
    maskd = np.ascontiguousarray(maskd)

    wq, wk, wv = w_attn[0:C], w_attn[C:2 * C], w_attn[2 * C:3 * C]
    in_maps = []
    for c in range(NCORES):
        h0, h1 = HPC * c, HPC * c + 1
        Wc = np.concatenate([
            _perm(wq[h0 * D:(h0 + 1) * D]), _perm(wq[h1 * D:(h1 + 1) * D]),
            _perm(wk[h0 * D:(h0 + 1) * D]), _perm(wk[h1 * D:(h1 + 1) * D]),
            wv[h0 * D:(h0 + 1) * D], wv[h1 * D:(h1 + 1) * D]], axis=0)
        wqkv_c = np.ascontiguousarray(
            Wc.T.reshape(KTC, P, FPC).transpose(1, 0, 2)
            .reshape(P, KTC * FPC).astype(f16))
        wp_c = w_proj[:, c * HPC * D:(c + 1) * HPC * D].T  # [256, C]
        wp_c = np.ascontiguousarray(
            wp_c.reshape(HPC, P, C).transpose(1, 0, 2)
            .reshape(P, HPC * C).astype(f16))
        in_maps.append({
            "xt": xT, "wqkv": wqkv_c, "cosw": cosw, "sinw": sinw,
            "maskd": maskd, "wproj": wp_c,
        })
    return in_maps


def kernel(x, mask, freqs_cos, freqs_sin, w_attn, w_proj):
    global _PROGRAM
    _ensure_concourse()
    from concourse.bass_utils import run_bass_kernel_spmd

    if _PROGRAM is None:
        _PROGRAM = _build_program()
    nc = _PROGRAM

    in_maps = _host_inputs(x, mask, freqs_cos, freqs_sin, w_attn, w_proj)
    res = run_bass_kernel_spmd(nc, in_maps, list(range(NCORES)))
    out = res.results[0]["outp"].astype(np.float64)
    for i in range(1, NCORES):
        out = out + res.results[i]["outp"]
    return np.ascontiguousarray(out.reshape(B, T, C).astype(np.float32))
